# revision 10
# baseline (speedup 1.0000x reference)
"""Trainium2 Bass kernel for nn_NeuralNetwork_65618510348896 (binarized MLP).

Network (training-mode BatchNorm, B=65536):
  h1 = x @ sign(W1).T + b1 ; h1c = clip(bn1(h1), -1, 1)
  h2 = sign(h1c) @ sign(W2).T + b2 ; h2c = clip(bn2(h2), -1, 1)
  h3 = sign(h2c) @ sign(W3).T + b3 ; h3c = clip(bn3(h3), -1, 1)
  out = log_softmax(h3c @ W4.T + b4)

Strategy: pure data parallel over 8 NeuronCores (8192 rows each), BatchNorm
statistics via AllReduce of per-core (sum h, sum h^2).

On-chip layout: feature-major activations [feat(part), batch(free)], weights
stationary as W^T.  fc1 runs in exact-fp16-split (x = x_hi + x_lo, both fp16;
sign weights are exact in fp16) giving ~2^-22 relative accuracy; fc2/fc3 are
sign x sign matmuls -> exact integer arithmetic in fp16/fp32 PSUM.
sign(clip(bn(h))) == sign(scale*h + bias) with per-feature scale/bias computed
from the all-reduced statistics, fused into one ScalarE activation per tile.
"""
import sys
sys.path.insert(0, "/opt/trn_rl_repo")
sys.path.insert(0, "/root/.axon_site/_ro/trn_rl_repo")

import numpy as np

N_CORES = 8
B_TOT = 65536
BC = B_TOT // N_CORES          # rows per core
NB = 512                       # batch block (moving free dim)
FIN1 = 896                     # 784 padded to 7*128
F1 = 512                       # 500 padded
F2 = 1024
F3 = 1024
F4 = 16                        # 10 padded
BN_EPS = 1e-5

_CACHE = {}


def build(nblk, phases=4):
    import concourse.bass as bass
    import concourse.mybir as mybir
    import concourse.tile as tile
    from concourse import bacc
    from concourse.masks import make_identity

    f32 = mybir.dt.float32
    f16 = mybir.dt.float16
    AF = mybir.ActivationFunctionType
    OP = mybir.AluOpType
    bc = nblk * NB

    nc = bacc.Bacc("TRN2", target_bir_lowering=False, debug=False,
                   num_devices=N_CORES)

    xhi_t = nc.dram_tensor("xhi", [bc, FIN1], f16, kind="ExternalInput")
    xlo_t = nc.dram_tensor("xlo", [bc, FIN1], f16, kind="ExternalInput")
    w1_t = nc.dram_tensor("sw1", [F1, FIN1], f16, kind="ExternalInput")
    w2_t = nc.dram_tensor("sw2", [F2, F1], f16, kind="ExternalInput")
    w3_t = nc.dram_tensor("sw3", [F3, F3], f16, kind="ExternalInput")
    w4_t = nc.dram_tensor("w4h", [F4, F3], f16, kind="ExternalInput")
    vec_names = ["b1", "g1", "be1", "b2", "g2", "be2", "b3", "g3", "be3"]
    vec_sizes = [F1, F1, F1, F2, F2, F2, F3, F3, F3]
    vec_t = {n: nc.dram_tensor(n, [s], f32, kind="ExternalInput")
             for n, s in zip(vec_names, vec_sizes)}
    b4_t = nc.dram_tensor("b4", [F4], f32, kind="ExternalInput")
    out_t = nc.dram_tensor("out", [bc, 16], f32, kind="ExternalOutput")

    K1, M1, M2, M3 = FIN1 // 128, F1 // 128, F2 // 128, F3 // 128

    with tile.TileContext(nc) as tc:
        import contextlib
        ctx = contextlib.ExitStack()
        with ctx:
            big = ctx.enter_context(tc.tile_pool(name="big", bufs=65))
            xtp = ctx.enter_context(tc.tile_pool(name="xtp", bufs=8))
            stp = ctx.enter_context(tc.tile_pool(name="stp", bufs=14))
            hfp = ctx.enter_context(tc.tile_pool(name="hfp", bufs=2))
            wp = ctx.enter_context(tc.tile_pool(name="wp", bufs=1))
            cst = ctx.enter_context(tc.tile_pool(name="cst", bufs=1))
            sml = ctx.enter_context(tc.tile_pool(name="sml", bufs=8))
            psp = ctx.enter_context(tc.tile_pool(name="psp", bufs=4, space="PSUM"))
            ps4 = ctx.enter_context(tc.tile_pool(name="ps4", bufs=2, space="PSUM"))
            pst = ctx.enter_context(tc.tile_pool(name="pstp", bufs=2, space="PSUM"))
            drp = ctx.enter_context(tc.tile_pool(name="drp", bufs=1, space="DRAM"))

            # ---------------- weight / vector prep ----------------
            # weights arrive as sign(W) (or cast W4) in fp16; transpose
            # directly from DRAM via the xbar into W^T tiles [128, fout].
            def load_wT(w_dram, fout, fin, name):
                nk = fin // 128
                wT = []
                for k in range(nk):
                    t = wp.tile([128, fout], f16, name=f"{name}_{k}",
                                tag=f"{name}_{k}")
                    nc.sync.dma_start_transpose(
                        t, w_dram[0:fout, k * 128:(k + 1) * 128])
                    wT.append(t)
                return wT

            w1T = load_wT(w1_t, F1, FIN1, "w1T")
            w2T = load_wT(w2_t, F2, F1, "w2T")
            w3T = load_wT(w3_t, F3, F3, "w3T")
            w4T = load_wT(w4_t, F4, F3, "w4T")

            def load_vec(name, nchunk):
                v = cst.tile([128, nchunk], f32, name=f"v_{name}", tag=f"v_{name}")
                nc.sync.dma_start(
                    v, vec_t[name].rearrange("(c p) -> p c", p=128))
                return v

            vb1, vg1, vbe1 = (load_vec(n, M1) for n in ("b1", "g1", "be1"))
            vb2, vg2, vbe2 = (load_vec(n, M2) for n in ("b2", "g2", "be2"))
            vb3, vg3, vbe3 = (load_vec(n, M3) for n in ("b3", "g3", "be3"))
            vb4 = cst.tile([16, 1], f32)
            nc.sync.dma_start(vb4, b4_t.rearrange("(p o) -> p o", o=1))
            ident = cst.tile([16, 16], f32)
            make_identity(nc, ident)

            # stats accumulators (sum h, and sum h^2 in the q tiles)
            s1cols = cst.tile([128, M1 * nblk], f32)
            s2cols = cst.tile([128, M2 * nblk], f32)
            s3cols = cst.tile([128, M3 * nblk], f32)
            s1colsq = cst.tile([128, M1 * nblk], f32)
            s2colsq = cst.tile([128, M2 * nblk], f32)
            s3colsq = cst.tile([128, M3 * nblk], f32)
            st1 = cst.tile([128, 2 * M1], f32)
            st2 = cst.tile([128, 2 * M2], f32)
            st3 = cst.tile([128, 2 * M3], f32)

            # DRAM scratch for h2 spill
            h2d = drp.tile([M2, 128, bc], f16)
            # collective bounce buffers
            ar1i = drp.tile([128, 2 * M1], f32)
            ar1o = drp.tile([128, 2 * M1], f32, addr_space="Shared")
            ar2i = drp.tile([128, 2 * M2], f32)
            ar2o = drp.tile([128, 2 * M2], f32, addr_space="Shared")
            ar3i = drp.tile([128, 2 * M3], f32)
            ar3o = drp.tile([128, 2 * M3], f32, addr_space="Shared")

            # ---------------- phase 1: fc1 + stats ----------------
            h1tiles = [[None] * M1 for _ in range(nblk)]
            for b in range(nblk):
                pss = [psp.tile([128, NB], f32, name="ps1", tag="mm")
                       for _ in range(M1)]
                for k in range(K1):
                    th = xtp.tile([128, NB], f16, name="xth", tag="xt")
                    nc.sync.dma_start_transpose(
                        th, xhi_t[b * NB:(b + 1) * NB,
                                  k * 128:(k + 1) * 128])
                    tl = xtp.tile([128, NB], f16, name="xtl", tag="xt")
                    nc.sync.dma_start_transpose(
                        tl, xlo_t[b * NB:(b + 1) * NB,
                                  k * 128:(k + 1) * 128])
                    for m in range(M1):
                        lhs = w1T[k][:, m * 128:(m + 1) * 128]
                        nc.tensor.matmul(pss[m], lhs, th,
                                         start=(k == 0), stop=False)
                        nc.tensor.matmul(pss[m], lhs, tl,
                                         start=False, stop=(k == K1 - 1))
                for m in range(M1):
                    h1 = big.tile([128, NB], f32, name="h1", tag="hb")
                    c = m * nblk + b
                    nc.vector.tensor_scalar(
                        out=h1, in0=pss[m], scalar1=0.0, scalar2=None,
                        op0=OP.add, op1=OP.add,
                        accum_out=s1cols[:, c:c + 1])
                    sq = stp.tile([128, NB], mybir.dt.bfloat16, name="sq",
                                  tag="sq", bufs=3)
                    nc.scalar.activation(sq, pss[m], AF.Square,
                                         accum_out=s1colsq[:, c:c + 1])
                    h1tiles[b][m] = h1

            # ---------------- allreduce 1 ----------------
            def do_allreduce(scols, scolsq, st, ari, aro, nch, gv, bev,
                             bv, nm):
                for m in range(nch):
                    nc.vector.tensor_reduce(
                        out=st[:, m:m + 1],
                        in_=scols[:, m * nblk:(m + 1) * nblk],
                        axis=mybir.AxisListType.X, op=OP.add)
                    nc.vector.tensor_reduce(
                        out=st[:, nch + m:nch + m + 1],
                        in_=scolsq[:, m * nblk:(m + 1) * nblk],
                        axis=mybir.AxisListType.X, op=OP.add)
                nc.sync.dma_start(ari, st)
                nc.gpsimd.collective_compute(
                    "AllReduce", OP.add,
                    replica_groups=[list(range(N_CORES))],
                    ins=[ari.opt()], outs=[aro.opt()])
                stg = cst.tile([128, 2 * nch], f32, name=f"stg{nm}",
                               tag=f"stg{nm}")
                nc.sync.dma_start(stg, aro)
                inv_b = 1.0 / (N_CORES * nblk * NB)
                mean = cst.tile([128, nch], f32, name=f"mean{nm}",
                                tag=f"mean{nm}")
                nc.vector.tensor_scalar_mul(mean, stg[:, 0:nch], inv_b)
                var = cst.tile([128, nch], f32, name=f"var{nm}",
                               tag=f"var{nm}")
                # var = E[h^2] - mean^2 ; then + eps
                nc.vector.tensor_scalar_mul(var, stg[:, nch:2 * nch], inv_b)
                msq = cst.tile([128, nch], f32, name=f"msq{nm}",
                               tag=f"msq{nm}")
                nc.vector.tensor_tensor(out=msq, in0=mean, in1=mean,
                                        op=OP.mult)
                nc.vector.tensor_tensor(out=var, in0=var, in1=msq,
                                        op=OP.subtract)
                nc.vector.tensor_scalar_add(var, var, BN_EPS)
                nc.vector.reciprocal(var, var)      # 1/(var+eps)
                rstd = cst.tile([128, nch], f32, name=f"rstd{nm}",
                                tag=f"rstd{nm}")
                nc.scalar.sqrt(rstd, var)           # 1/sqrt(var+eps)
                sc = cst.tile([128, nch], f32, name=f"sc{nm}", tag=f"sc{nm}")
                nc.vector.tensor_tensor(out=sc, in0=gv, in1=rstd, op=OP.mult)
                # bias = be - mean*sc  (the fc bias cancels under
                # training-mode BN: mean_true = mean_nobias + b)
                bi = cst.tile([128, nch], f32, name=f"bi{nm}", tag=f"bi{nm}")
                nc.vector.tensor_tensor(out=bi, in0=mean, in1=sc, op=OP.mult)
                nc.vector.tensor_tensor(out=bi, in0=bev, in1=bi,
                                        op=OP.subtract)
                return sc, bi

            sc1, bi1 = do_allreduce(s1cols, s1colsq, st1, ar1i, ar1o, M1,
                                    vg1, vbe1, vb1, 1)

            # ---------------- phase 2: sign1 + fc2 + stats, spill ----------
            for b in range(nblk if phases >= 2 else 0):
                sh1 = []
                for m in range(M1):
                    s = stp.tile([128, NB], f16, name="sh1", tag="st")
                    nc.scalar.activation(s, h1tiles[b][m], AF.Sign,
                                         bias=bi1[:, m:m + 1],
                                         scale=sc1[:, m:m + 1])
                    sh1.append(s)
                for j in range(M2):
                    ps = psp.tile([128, NB], f32, name="ps2", tag="mm")
                    for k in range(M1):
                        nc.tensor.matmul(ps, w2T[k][:, j * 128:(j + 1) * 128],
                                         sh1[k], start=(k == 0),
                                         stop=(k == M1 - 1))
                    h2 = stp.tile([128, NB], f16, name="h2", tag="st")
                    c = j * nblk + b
                    nc.vector.tensor_scalar(
                        out=h2, in0=ps, scalar1=0.0, scalar2=None,
                        op0=OP.add, op1=OP.add,
                        accum_out=s2cols[:, c:c + 1])
                    sq = stp.tile([128, NB], mybir.dt.bfloat16, name="sq",
                                  tag="sq", bufs=3)
                    nc.scalar.activation(sq, ps, AF.Square,
                                         accum_out=s2colsq[:, c:c + 1])
                    nc.sync.dma_start(h2d[j, :, b * NB:(b + 1) * NB], h2)

            if phases >= 2:
                sc2, bi2 = do_allreduce(s2cols, s2colsq, st2, ar2i, ar2o, M2,
                                        vg2, vbe2, vb2, 2)

            # ---------------- phase 3: sign2 + fc3 + stats ----------------
            h3tiles = [[None] * M3 for _ in range((nblk + 1) // 2)]
            for b in range(nblk if phases >= 3 else 0):
                sh2 = []
                for k in range(M2):
                    r2 = stp.tile([128, NB], f16, name="r2", tag="st")
                    nc.sync.dma_start(r2, h2d[k, :, b * NB:(b + 1) * NB])
                    s = stp.tile([128, NB], f16, name="sh2", tag="st")
                    nc.scalar.activation(s, r2, AF.Sign,
                                         bias=bi2[:, k:k + 1],
                                         scale=sc2[:, k:k + 1])
                    sh2.append(s)
                for j in range(M3):
                    ps = psp.tile([128, NB], f32, name="ps3", tag="mm")
                    for k in range(M2):
                        nc.tensor.matmul(ps, w3T[k][:, j * 128:(j + 1) * 128],
                                         sh2[k], start=(k == 0),
                                         stop=(k == M2 - 1))
                    if b % 2 == 0:
                        h3tiles[b // 2][j] = big.tile(
                            [128, 2 * NB], f16, name="h3", tag="hb")
                    h3 = h3tiles[b // 2][j]
                    half = h3[:, (b % 2) * NB:(b % 2 + 1) * NB]
                    c = j * nblk + b
                    nc.vector.tensor_scalar(
                        out=half, in0=ps, scalar1=0.0, scalar2=None,
                        op0=OP.add, op1=OP.add,
                        accum_out=s3cols[:, c:c + 1])
                    sq = stp.tile([128, NB], mybir.dt.bfloat16, name="sq",
                                  tag="sq", bufs=3)
                    nc.scalar.activation(sq, ps, AF.Square,
                                         accum_out=s3colsq[:, c:c + 1])

            if phases >= 3:
                sc3, bi3 = do_allreduce(s3cols, s3colsq, st3, ar3i, ar3o, M3,
                                        vg3, vbe3, vb3, 3)

            # ---------------- phase 4: clip3 + fc4 + log_softmax ----------
            for b in range(nblk if phases >= 4 else 0):
                ch3 = []
                for k in range(M3):
                    half = h3tiles[b // 2][k][:, (b % 2) * NB:(b % 2 + 1) * NB]
                    t = stp.tile([128, NB], f16, name="t3", tag="st")
                    nc.scalar.activation(t, half, AF.Identity,
                                         bias=bi3[:, k:k + 1],
                                         scale=sc3[:, k:k + 1])
                    cl = stp.tile([128, NB], f16, name="cl3", tag="st")
                    nc.vector.tensor_scalar(out=cl, in0=t, scalar1=-1.0,
                                            scalar2=1.0, op0=OP.max,
                                            op1=OP.min)
                    ch3.append(cl)
                ps_l = ps4.tile([16, NB], f32, name="psl", tag="psl")
                for k in range(M3):
                    nc.tensor.matmul(ps_l, w4T[k], ch3[k],
                                     start=(k == 0), stop=(k == M3 - 1))
                lg = hfp.tile([16, NB], f32, name="lg", tag="hf")
                nc.scalar.activation(lg, ps_l, AF.Identity, bias=vb4)
                for r in range(4):
                    pt = pst.tile([128, 16], f32, name="pt", tag="pt")
                    nc.tensor.transpose(pt, lg[:, r * 128:(r + 1) * 128],
                                        ident)
                    e = sml.tile([128, 16], f32, name="e", tag="sm")
                    nc.scalar.activation(e[:, 0:10], pt[:, 0:10], AF.Exp)
                    se = sml.tile([128, 1], f32, name="se", tag="se")
                    nc.vector.tensor_reduce(out=se, in_=e[:, 0:10],
                                            axis=mybir.AxisListType.X,
                                            op=OP.add)
                    ls = sml.tile([128, 1], f32, name="ls", tag="ls")
                    nc.scalar.activation(ls, se, AF.Ln)
                    o = sml.tile([128, 16], f32, name="o", tag="sm")
                    nc.vector.tensor_scalar(out=o[:, 0:10], in0=pt[:, 0:10],
                                            scalar1=ls, scalar2=None,
                                            op0=OP.subtract)
                    nc.sync.dma_start(
                        out_t[b * NB + r * 128:b * NB + (r + 1) * 128, 0:10],
                        o[:, 0:10])

    nc.compile()
    return nc


def _pad(a, shape):
    out = np.zeros(shape, a.dtype)
    out[tuple(slice(0, s) for s in a.shape)] = a
    return out


def prepare_inputs(x, W1, b1, g1, be1, W2, b2, g2, be2, W3, b3, g3, be3,
                   W4, b4, nblk):
    bc = nblk * NB
    n = x.shape[0] // N_CORES
    common = {
        "sw1": _pad(np.sign(np.asarray(W1, np.float32)).astype(np.float16),
                    (F1, FIN1)),
        "sw2": _pad(np.sign(np.asarray(W2, np.float32)).astype(np.float16),
                    (F2, F1)),
        "sw3": np.sign(np.asarray(W3, np.float32)).astype(np.float16),
        "w4h": _pad(np.asarray(W4, np.float32).astype(np.float16), (F4, F3)),
        "b1": _pad(np.asarray(b1, np.float32), (F1,)),
        "g1": _pad(np.asarray(g1, np.float32), (F1,)),
        "be1": _pad(np.asarray(be1, np.float32), (F1,)),
        "b2": np.asarray(b2, np.float32), "g2": np.asarray(g2, np.float32),
        "be2": np.asarray(be2, np.float32),
        "b3": np.asarray(b3, np.float32), "g3": np.asarray(g3, np.float32),
        "be3": np.asarray(be3, np.float32),
        "b4": _pad(np.asarray(b4, np.float32), (F4,)),
    }
    xf = np.asarray(x, np.float32)
    xhi = xf.astype(np.float16)
    xlo = (xf - xhi.astype(np.float32)).astype(np.float16)
    xhi = _pad(xhi, (x.shape[0], FIN1))
    xlo = _pad(xlo, (x.shape[0], FIN1))
    return [dict(common, xhi=xhi[i * n:(i + 1) * n],
                 xlo=xlo[i * n:(i + 1) * n]) for i in range(N_CORES)]


class SpmdRunner:
    """Build-once/run-many executor via PJRT (adapted from
    concourse.bass2jax.run_bass_via_pjrt)."""

    def __init__(self, nc, n_cores):
        import jax
        import concourse.mybir as mybir
        from concourse import bass2jax
        from concourse.bass2jax import _bass_exec_p, install_neuronx_cc_hook
        from jax.sharding import Mesh, PartitionSpec
        from jax.experimental.shard_map import shard_map

        install_neuronx_cc_hook()
        self.jax = jax
        self.nc = nc
        self.n_cores = n_cores
        partition_name = (nc.partition_id_tensor.name
                          if nc.partition_id_tensor else None)
        in_names, out_names, out_avals, zero_outs = [], [], [], []
        for alloc in nc.m.functions[0].allocations:
            if not isinstance(alloc, mybir.MemoryLocationSet):
                continue
            name = alloc.memorylocations[0].name
            if alloc.kind == "ExternalInput":
                if name != partition_name:
                    in_names.append(name)
            elif alloc.kind == "ExternalOutput":
                out_names.append(name)
                shape = tuple(alloc.tensor_shape)
                dtype = mybir.dt.np(alloc.dtype)
                out_avals.append(jax.core.ShapedArray(shape, dtype))
                zero_outs.append(np.zeros(shape, dtype))
        self.in_names = list(in_names)
        self.out_names = out_names
        self.out_avals = out_avals
        self.zero_outs = zero_outs
        n_params = len(in_names)
        n_outs = len(out_avals)
        all_in_names = list(in_names) + list(out_names)
        if partition_name is not None:
            all_in_names.append(partition_name)

        def _body(*args):
            operands = list(args)
            if partition_name is not None:
                operands.append(bass2jax.partition_id_tensor())
            outs = _bass_exec_p.bind(
                *operands,
                out_avals=tuple(out_avals),
                in_names=tuple(all_in_names),
                out_names=tuple(out_names),
                lowering_input_output_aliases=(),
                sim_require_finite=True,
                sim_require_nnan=True,
                nc=nc,
            )
            return tuple(outs)

        devices = jax.devices()[:n_cores]
        mesh = Mesh(np.asarray(devices), ("core",))
        in_specs = (PartitionSpec("core"),) * (n_params + n_outs)
        out_specs = (PartitionSpec("core"),) * n_outs
        self.sharded = jax.jit(
            shard_map(_body, mesh=mesh, in_specs=in_specs,
                      out_specs=out_specs, check_rep=False),
            keep_unused=True,
        )

    def prepare(self, in_maps):
        n = self.n_cores
        args = []
        for name in self.in_names:
            args.append(np.concatenate(
                [np.asarray(in_maps[c][name]) for c in range(n)], axis=0))
        for z in self.zero_outs:
            args.append(np.zeros((n * z.shape[0], *z.shape[1:]), z.dtype))
        return [self.jax.device_put(a) for a in args]

    def run(self, dev_args):
        outs = self.sharded(*dev_args)
        self.jax.block_until_ready(outs)
        return outs

    def results(self, outs):
        res = []
        for c in range(self.n_cores):
            d = {}
            for i, name in enumerate(self.out_names):
                d[name] = np.asarray(outs[i]).reshape(
                    self.n_cores, *self.out_avals[i].shape)[c]
            res.append(d)
        return res

    def time_runs(self, dev_args, iters=5, warmup=2):
        import time
        for _ in range(warmup):
            self.run(dev_args)
        ts = []
        for _ in range(iters):
            t0 = time.perf_counter()
            self.run(dev_args)
            ts.append(time.perf_counter() - t0)
        return min(ts)


def get_runner(nblk=BC // NB):
    if nblk not in _CACHE:
        nc = build(nblk)
        _CACHE[nblk] = SpmdRunner(nc, N_CORES)
    return _CACHE[nblk]


def kernel(**inputs) -> np.ndarray:
    r = get_runner()
    in_maps = prepare_inputs(nblk=BC // NB, **inputs)
    dev = r.prepare(in_maps)
    outs = r.run(dev)
    res = r.results(outs)
    return np.concatenate([res[i]["out"][:, 0:10] for i in range(N_CORES)],
                          axis=0)


# revision 11
# speedup vs baseline: 21.7909x; 21.7909x over previous
"""Trainium2 Bass kernel for nn_NeuralNetwork_65618510348896 (binarized MLP).

Network (training-mode BatchNorm, B=65536):
  h1 = x @ sign(W1).T + b1 ; h1c = clip(bn1(h1), -1, 1)
  h2 = sign(h1c) @ sign(W2).T + b2 ; h2c = clip(bn2(h2), -1, 1)
  h3 = sign(h2c) @ sign(W3).T + b3 ; h3c = clip(bn3(h3), -1, 1)
  out = log_softmax(h3c @ W4.T + b4)

Strategy: pure data parallel over 8 NeuronCores (8192 rows each), BatchNorm
statistics via AllReduce of per-core (sum h, sum h^2).

On-chip layout: feature-major activations [feat(part), batch(free)], weights
stationary as W^T.  fc1 runs in exact-fp16-split (x = x_hi + x_lo, both fp16;
sign weights are exact in fp16) giving ~2^-22 relative accuracy; fc2/fc3 are
sign x sign matmuls -> exact integer arithmetic in fp16/fp32 PSUM.
sign(clip(bn(h))) == sign(scale*h + bias) with per-feature scale/bias computed
from the all-reduced statistics, fused into one ScalarE activation per tile.
"""
import sys
sys.path.insert(0, "/opt/trn_rl_repo")
sys.path.insert(0, "/root/.axon_site/_ro/trn_rl_repo")

import numpy as np

N_CORES = 8
B_TOT = 65536
BC = B_TOT // N_CORES          # rows per core
NB = 512                       # batch block (moving free dim)
FIN1 = 896                     # 784 padded to 7*128
F1 = 512                       # 500 padded
F2 = 1024
F3 = 1024
F4 = 16                        # 10 padded
BN_EPS = 1e-5

_CACHE = {}


def build(nblk, phases=4):
    import concourse.bass as bass
    import concourse.mybir as mybir
    import concourse.tile as tile
    from concourse import bacc
    from concourse.masks import make_identity

    f32 = mybir.dt.float32
    f16 = mybir.dt.float16
    AF = mybir.ActivationFunctionType
    OP = mybir.AluOpType
    bc = nblk * NB

    nc = bacc.Bacc("TRN2", target_bir_lowering=False, debug=False,
                   num_devices=N_CORES)

    xhi_t = nc.dram_tensor("xhi", [bc, FIN1], f16, kind="ExternalInput")
    xlo_t = nc.dram_tensor("xlo", [bc, FIN1], f16, kind="ExternalInput")
    w1_t = nc.dram_tensor("sw1", [F1, FIN1], f16, kind="ExternalInput")
    w2_t = nc.dram_tensor("sw2", [F2, F1], f16, kind="ExternalInput")
    w3_t = nc.dram_tensor("sw3", [F3, F3], f16, kind="ExternalInput")
    w4_t = nc.dram_tensor("w4h", [F4, F3], f16, kind="ExternalInput")
    vec_names = ["b1", "g1", "be1", "b2", "g2", "be2", "b3", "g3", "be3"]
    vec_sizes = [F1, F1, F1, F2, F2, F2, F3, F3, F3]
    vec_t = {n: nc.dram_tensor(n, [s], f32, kind="ExternalInput")
             for n, s in zip(vec_names, vec_sizes)}
    b4_t = nc.dram_tensor("b4", [F4], f32, kind="ExternalInput")
    out_t = nc.dram_tensor("out", [bc, 16], f32, kind="ExternalOutput")

    K1, M1, M2, M3 = FIN1 // 128, F1 // 128, F2 // 128, F3 // 128

    with tile.TileContext(nc) as tc:
        import contextlib
        ctx = contextlib.ExitStack()
        with ctx:
            big = ctx.enter_context(tc.tile_pool(name="big", bufs=65))
            xtp = ctx.enter_context(tc.tile_pool(name="xtp", bufs=8))
            stp = ctx.enter_context(tc.tile_pool(name="stp", bufs=14))
            hfp = ctx.enter_context(tc.tile_pool(name="hfp", bufs=2))
            wp = ctx.enter_context(tc.tile_pool(name="wp", bufs=1))
            cst = ctx.enter_context(tc.tile_pool(name="cst", bufs=1))
            sml = ctx.enter_context(tc.tile_pool(name="sml", bufs=8))
            psp = ctx.enter_context(tc.tile_pool(name="psp", bufs=4, space="PSUM"))
            ps4 = ctx.enter_context(tc.tile_pool(name="ps4", bufs=2, space="PSUM"))
            pst = ctx.enter_context(tc.tile_pool(name="pstp", bufs=2, space="PSUM"))
            drp = ctx.enter_context(tc.tile_pool(name="drp", bufs=1, space="DRAM"))

            # ---------------- weight / vector prep ----------------
            # weights arrive as sign(W) (or cast W4) in fp16; transpose
            # directly from DRAM via the xbar into W^T tiles [128, fout].
            def load_wT(w_dram, fout, fin, name):
                nk = fin // 128
                wT = []
                for k in range(nk):
                    t = wp.tile([128, fout], f16, name=f"{name}_{k}",
                                tag=f"{name}_{k}")
                    nc.sync.dma_start_transpose(
                        t, w_dram[0:fout, k * 128:(k + 1) * 128])
                    wT.append(t)
                return wT

            w1T = load_wT(w1_t, F1, FIN1, "w1T")
            w2T = load_wT(w2_t, F2, F1, "w2T")
            w3T = load_wT(w3_t, F3, F3, "w3T")
            w4T = load_wT(w4_t, F4, F3, "w4T")

            def load_vec(name, nchunk):
                v = cst.tile([128, nchunk], f32, name=f"v_{name}", tag=f"v_{name}")
                nc.sync.dma_start(
                    v, vec_t[name].rearrange("(c p) -> p c", p=128))
                return v

            vb1, vg1, vbe1 = (load_vec(n, M1) for n in ("b1", "g1", "be1"))
            vb2, vg2, vbe2 = (load_vec(n, M2) for n in ("b2", "g2", "be2"))
            vb3, vg3, vbe3 = (load_vec(n, M3) for n in ("b3", "g3", "be3"))
            vb4 = cst.tile([16, 1], f32)
            nc.sync.dma_start(vb4, b4_t.rearrange("(p o) -> p o", o=1))
            ident = cst.tile([16, 16], f32)
            make_identity(nc, ident)

            # stats accumulators (sum h, and sum h^2 in the q tiles)
            s1cols = cst.tile([128, M1 * nblk], f32)
            s2cols = cst.tile([128, M2 * nblk], f32)
            s3cols = cst.tile([128, M3 * nblk], f32)
            s1colsq = cst.tile([128, M1 * nblk], f32)
            s2colsq = cst.tile([128, M2 * nblk], f32)
            s3colsq = cst.tile([128, M3 * nblk], f32)
            st1 = cst.tile([128, 2 * M1], f32)
            st2 = cst.tile([128, 2 * M2], f32)
            st3 = cst.tile([128, 2 * M3], f32)

            # DRAM scratch for h2 spill
            h2d = drp.tile([M2, 128, bc], f16)
            # collective bounce buffers
            ar1i = drp.tile([128, 2 * M1], f32)
            ar1o = drp.tile([128, 2 * M1], f32, addr_space="Shared")
            ar2i = drp.tile([128, 2 * M2], f32)
            ar2o = drp.tile([128, 2 * M2], f32, addr_space="Shared")
            ar3i = drp.tile([128, 2 * M3], f32)
            ar3o = drp.tile([128, 2 * M3], f32, addr_space="Shared")

            # ---------------- phase 1: fc1 + stats ----------------
            h1tiles = [[None] * M1 for _ in range(nblk)]
            for b in range(nblk):
                pss = [psp.tile([128, NB], f32, name="ps1", tag="mm")
                       for _ in range(M1)]
                for k in range(K1):
                    th = xtp.tile([128, NB], f16, name="xth", tag="xt")
                    nc.sync.dma_start_transpose(
                        th, xhi_t[b * NB:(b + 1) * NB,
                                  k * 128:(k + 1) * 128])
                    tl = xtp.tile([128, NB], f16, name="xtl", tag="xt")
                    nc.sync.dma_start_transpose(
                        tl, xlo_t[b * NB:(b + 1) * NB,
                                  k * 128:(k + 1) * 128])
                    for m in range(M1):
                        lhs = w1T[k][:, m * 128:(m + 1) * 128]
                        nc.tensor.matmul(pss[m], lhs, th,
                                         start=(k == 0), stop=False)
                        nc.tensor.matmul(pss[m], lhs, tl,
                                         start=False, stop=(k == K1 - 1))
                for m in range(M1):
                    h1 = big.tile([128, NB], f32, name="h1", tag="hb")
                    c = m * nblk + b
                    nc.vector.tensor_scalar(
                        out=h1, in0=pss[m], scalar1=0.0, scalar2=None,
                        op0=OP.add, op1=OP.add,
                        accum_out=s1cols[:, c:c + 1])
                    sq = stp.tile([128, NB], mybir.dt.bfloat16, name="sq",
                                  tag="sq", bufs=3)
                    nc.scalar.activation(sq, pss[m], AF.Square,
                                         accum_out=s1colsq[:, c:c + 1])
                    h1tiles[b][m] = h1

            # ---------------- allreduce 1 ----------------
            def do_allreduce(scols, scolsq, st, ari, aro, nch, gv, bev,
                             bv, nm):
                for m in range(nch):
                    nc.vector.tensor_reduce(
                        out=st[:, m:m + 1],
                        in_=scols[:, m * nblk:(m + 1) * nblk],
                        axis=mybir.AxisListType.X, op=OP.add)
                    nc.vector.tensor_reduce(
                        out=st[:, nch + m:nch + m + 1],
                        in_=scolsq[:, m * nblk:(m + 1) * nblk],
                        axis=mybir.AxisListType.X, op=OP.add)
                nc.sync.dma_start(ari, st)
                nc.gpsimd.collective_compute(
                    "AllReduce", OP.add,
                    replica_groups=[list(range(N_CORES))],
                    ins=[ari.opt()], outs=[aro.opt()])
                stg = cst.tile([128, 2 * nch], f32, name=f"stg{nm}",
                               tag=f"stg{nm}")
                nc.sync.dma_start(stg, aro)
                inv_b = 1.0 / (N_CORES * nblk * NB)
                mean = cst.tile([128, nch], f32, name=f"mean{nm}",
                                tag=f"mean{nm}")
                nc.vector.tensor_scalar_mul(mean, stg[:, 0:nch], inv_b)
                var = cst.tile([128, nch], f32, name=f"var{nm}",
                               tag=f"var{nm}")
                # var = E[h^2] - mean^2 ; then + eps
                nc.vector.tensor_scalar_mul(var, stg[:, nch:2 * nch], inv_b)
                msq = cst.tile([128, nch], f32, name=f"msq{nm}",
                               tag=f"msq{nm}")
                nc.vector.tensor_tensor(out=msq, in0=mean, in1=mean,
                                        op=OP.mult)
                nc.vector.tensor_tensor(out=var, in0=var, in1=msq,
                                        op=OP.subtract)
                nc.vector.tensor_scalar_add(var, var, BN_EPS)
                nc.vector.reciprocal(var, var)      # 1/(var+eps)
                rstd = cst.tile([128, nch], f32, name=f"rstd{nm}",
                                tag=f"rstd{nm}")
                nc.scalar.sqrt(rstd, var)           # 1/sqrt(var+eps)
                sc = cst.tile([128, nch], f32, name=f"sc{nm}", tag=f"sc{nm}")
                nc.vector.tensor_tensor(out=sc, in0=gv, in1=rstd, op=OP.mult)
                # bias = be - mean*sc  (the fc bias cancels under
                # training-mode BN: mean_true = mean_nobias + b)
                bi = cst.tile([128, nch], f32, name=f"bi{nm}", tag=f"bi{nm}")
                nc.vector.tensor_tensor(out=bi, in0=mean, in1=sc, op=OP.mult)
                nc.vector.tensor_tensor(out=bi, in0=bev, in1=bi,
                                        op=OP.subtract)
                return sc, bi

            sc1, bi1 = do_allreduce(s1cols, s1colsq, st1, ar1i, ar1o, M1,
                                    vg1, vbe1, vb1, 1)

            # ---------------- phase 2: sign1 + fc2 + stats, spill ----------
            for b in range(nblk if phases >= 2 else 0):
                sh1 = []
                for m in range(M1):
                    s = stp.tile([128, NB], f16, name="sh1", tag="st")
                    nc.scalar.activation(s, h1tiles[b][m], AF.Sign,
                                         bias=bi1[:, m:m + 1],
                                         scale=sc1[:, m:m + 1])
                    sh1.append(s)
                for j in range(M2):
                    ps = psp.tile([128, NB], f32, name="ps2", tag="mm")
                    for k in range(M1):
                        nc.tensor.matmul(ps, w2T[k][:, j * 128:(j + 1) * 128],
                                         sh1[k], start=(k == 0),
                                         stop=(k == M1 - 1))
                    h2 = stp.tile([128, NB], f16, name="h2", tag="st")
                    c = j * nblk + b
                    nc.vector.tensor_scalar(
                        out=h2, in0=ps, scalar1=0.0, scalar2=None,
                        op0=OP.add, op1=OP.add,
                        accum_out=s2cols[:, c:c + 1])
                    sq = stp.tile([128, NB], mybir.dt.bfloat16, name="sq",
                                  tag="sq", bufs=3)
                    nc.scalar.activation(sq, ps, AF.Square,
                                         accum_out=s2colsq[:, c:c + 1])
                    nc.sync.dma_start(h2d[j, :, b * NB:(b + 1) * NB], h2)

            if phases >= 2:
                sc2, bi2 = do_allreduce(s2cols, s2colsq, st2, ar2i, ar2o, M2,
                                        vg2, vbe2, vb2, 2)

            # ---------------- phase 3: sign2 + fc3 + stats ----------------
            h3tiles = [[None] * M3 for _ in range((nblk + 1) // 2)]
            for b in range(nblk if phases >= 3 else 0):
                sh2 = []
                for k in range(M2):
                    r2 = stp.tile([128, NB], f16, name="r2", tag="st")
                    nc.sync.dma_start(r2, h2d[k, :, b * NB:(b + 1) * NB])
                    s = stp.tile([128, NB], f16, name="sh2", tag="st")
                    nc.scalar.activation(s, r2, AF.Sign,
                                         bias=bi2[:, k:k + 1],
                                         scale=sc2[:, k:k + 1])
                    sh2.append(s)
                for j in range(M3):
                    ps = psp.tile([128, NB], f32, name="ps3", tag="mm")
                    for k in range(M2):
                        nc.tensor.matmul(ps, w3T[k][:, j * 128:(j + 1) * 128],
                                         sh2[k], start=(k == 0),
                                         stop=(k == M2 - 1))
                    if b % 2 == 0:
                        h3tiles[b // 2][j] = big.tile(
                            [128, 2 * NB], f16, name="h3", tag="hb")
                    h3 = h3tiles[b // 2][j]
                    half = h3[:, (b % 2) * NB:(b % 2 + 1) * NB]
                    c = j * nblk + b
                    nc.vector.tensor_scalar(
                        out=half, in0=ps, scalar1=0.0, scalar2=None,
                        op0=OP.add, op1=OP.add,
                        accum_out=s3cols[:, c:c + 1])
                    sq = stp.tile([128, NB], mybir.dt.bfloat16, name="sq",
                                  tag="sq", bufs=3)
                    nc.scalar.activation(sq, ps, AF.Square,
                                         accum_out=s3colsq[:, c:c + 1])

            if phases >= 3:
                sc3, bi3 = do_allreduce(s3cols, s3colsq, st3, ar3i, ar3o, M3,
                                        vg3, vbe3, vb3, 3)

            # ---------------- phase 4: clip3 + fc4 + log_softmax ----------
            for b in range(nblk if phases >= 4 else 0):
                ch3 = []
                for k in range(M3):
                    half = h3tiles[b // 2][k][:, (b % 2) * NB:(b % 2 + 1) * NB]
                    t = stp.tile([128, NB], f16, name="t3", tag="st")
                    nc.scalar.activation(t, half, AF.Identity,
                                         bias=bi3[:, k:k + 1],
                                         scale=sc3[:, k:k + 1])
                    cl = stp.tile([128, NB], f16, name="cl3", tag="st")
                    nc.vector.tensor_scalar(out=cl, in0=t, scalar1=-1.0,
                                            scalar2=1.0, op0=OP.max,
                                            op1=OP.min)
                    ch3.append(cl)
                ps_l = ps4.tile([16, NB], f32, name="psl", tag="psl")
                for k in range(M3):
                    nc.tensor.matmul(ps_l, w4T[k], ch3[k],
                                     start=(k == 0), stop=(k == M3 - 1))
                lg = hfp.tile([16, NB], f32, name="lg", tag="hf")
                nc.scalar.activation(lg, ps_l, AF.Identity, bias=vb4)
                for r in range(4):
                    pt = pst.tile([128, 16], f32, name="pt", tag="pt")
                    nc.tensor.transpose(pt, lg[:, r * 128:(r + 1) * 128],
                                        ident)
                    e = sml.tile([128, 16], f32, name="e", tag="sm")
                    nc.scalar.activation(e[:, 0:10], pt[:, 0:10], AF.Exp)
                    se = sml.tile([128, 1], f32, name="se", tag="se")
                    nc.vector.tensor_reduce(out=se, in_=e[:, 0:10],
                                            axis=mybir.AxisListType.X,
                                            op=OP.add)
                    ls = sml.tile([128, 1], f32, name="ls", tag="ls")
                    nc.scalar.activation(ls, se, AF.Ln)
                    o = sml.tile([128, 16], f32, name="o", tag="sm")
                    nc.vector.tensor_scalar(out=o[:, 0:10], in0=pt[:, 0:10],
                                            scalar1=ls, scalar2=None,
                                            op0=OP.subtract)
                    nc.sync.dma_start(
                        out_t[b * NB + r * 128:b * NB + (r + 1) * 128, 0:10],
                        o[:, 0:10])

    nc.compile()
    return nc


def _pad(a, shape):
    out = np.zeros(shape, a.dtype)
    out[tuple(slice(0, s) for s in a.shape)] = a
    return out


def prepare_inputs(x, W1, b1, g1, be1, W2, b2, g2, be2, W3, b3, g3, be3,
                   W4, b4, nblk):
    bc = nblk * NB
    n = x.shape[0] // N_CORES
    common = {
        "sw1": _pad(np.sign(np.asarray(W1, np.float32)).astype(np.float16),
                    (F1, FIN1)),
        "sw2": _pad(np.sign(np.asarray(W2, np.float32)).astype(np.float16),
                    (F2, F1)),
        "sw3": np.sign(np.asarray(W3, np.float32)).astype(np.float16),
        "w4h": _pad(np.asarray(W4, np.float32).astype(np.float16), (F4, F3)),
        "b1": _pad(np.asarray(b1, np.float32), (F1,)),
        "g1": _pad(np.asarray(g1, np.float32), (F1,)),
        "be1": _pad(np.asarray(be1, np.float32), (F1,)),
        "b2": np.asarray(b2, np.float32), "g2": np.asarray(g2, np.float32),
        "be2": np.asarray(be2, np.float32),
        "b3": np.asarray(b3, np.float32), "g3": np.asarray(g3, np.float32),
        "be3": np.asarray(be3, np.float32),
        "b4": _pad(np.asarray(b4, np.float32), (F4,)),
    }
    xf = np.asarray(x, np.float32)
    xhi = xf.astype(np.float16)
    xlo = (xf - xhi.astype(np.float32)).astype(np.float16)
    xhi = _pad(xhi, (x.shape[0], FIN1))
    xlo = _pad(xlo, (x.shape[0], FIN1))
    return [dict(common, xhi=xhi[i * n:(i + 1) * n],
                 xlo=xlo[i * n:(i + 1) * n]) for i in range(N_CORES)]


class SpmdRunner:
    """Build-once/run-many executor via PJRT (adapted from
    concourse.bass2jax.run_bass_via_pjrt)."""

    def __init__(self, nc, n_cores):
        import jax
        import concourse.mybir as mybir
        from concourse import bass2jax
        from concourse.bass2jax import _bass_exec_p, install_neuronx_cc_hook
        from jax.sharding import Mesh, PartitionSpec
        from jax.experimental.shard_map import shard_map

        install_neuronx_cc_hook()
        self.jax = jax
        self.nc = nc
        self.n_cores = n_cores
        partition_name = (nc.partition_id_tensor.name
                          if nc.partition_id_tensor else None)
        in_names, out_names, out_avals, zero_outs = [], [], [], []
        for alloc in nc.m.functions[0].allocations:
            if not isinstance(alloc, mybir.MemoryLocationSet):
                continue
            name = alloc.memorylocations[0].name
            if alloc.kind == "ExternalInput":
                if name != partition_name:
                    in_names.append(name)
            elif alloc.kind == "ExternalOutput":
                out_names.append(name)
                shape = tuple(alloc.tensor_shape)
                dtype = mybir.dt.np(alloc.dtype)
                out_avals.append(jax.core.ShapedArray(shape, dtype))
                zero_outs.append(np.zeros(shape, dtype))
        self.in_names = list(in_names)
        self.out_names = out_names
        self.out_avals = out_avals
        self.zero_outs = zero_outs
        n_params = len(in_names)
        n_outs = len(out_avals)
        all_in_names = list(in_names) + list(out_names)
        if partition_name is not None:
            all_in_names.append(partition_name)

        def _body(*args):
            operands = list(args)
            if partition_name is not None:
                operands.append(bass2jax.partition_id_tensor())
            outs = _bass_exec_p.bind(
                *operands,
                out_avals=tuple(out_avals),
                in_names=tuple(all_in_names),
                out_names=tuple(out_names),
                lowering_input_output_aliases=(),
                sim_require_finite=True,
                sim_require_nnan=True,
                nc=nc,
            )
            return tuple(outs)

        devices = jax.devices()[:n_cores]
        mesh = Mesh(np.asarray(devices), ("core",))
        self.mesh = mesh
        self.PartitionSpec = PartitionSpec
        in_specs = (PartitionSpec("core"),) * (n_params + n_outs)
        out_specs = (PartitionSpec("core"),) * n_outs
        self.sharded = jax.jit(
            shard_map(_body, mesh=mesh, in_specs=in_specs,
                      out_specs=out_specs, check_rep=False),
            keep_unused=True,
        )

    def prepare(self, in_maps):
        n = self.n_cores
        args = []
        for name in self.in_names:
            args.append(np.concatenate(
                [np.asarray(in_maps[c][name]) for c in range(n)], axis=0))
        for z in self.zero_outs:
            args.append(np.zeros((n * z.shape[0], *z.shape[1:]), z.dtype))
        from jax.sharding import NamedSharding
        sh = NamedSharding(self.mesh, self.PartitionSpec("core"))
        return [self.jax.device_put(a, sh) for a in args]

    def run(self, dev_args):
        outs = self.sharded(*dev_args)
        self.jax.block_until_ready(outs)
        return outs

    def results(self, outs):
        res = []
        for c in range(self.n_cores):
            d = {}
            for i, name in enumerate(self.out_names):
                d[name] = np.asarray(outs[i]).reshape(
                    self.n_cores, *self.out_avals[i].shape)[c]
            res.append(d)
        return res

    def time_runs(self, dev_args, iters=5, warmup=2):
        import time
        for _ in range(warmup):
            self.run(dev_args)
        ts = []
        for _ in range(iters):
            t0 = time.perf_counter()
            self.run(dev_args)
            ts.append(time.perf_counter() - t0)
        return min(ts)


def get_runner(nblk=BC // NB):
    if nblk not in _CACHE:
        nc = build(nblk)
        _CACHE[nblk] = SpmdRunner(nc, N_CORES)
    return _CACHE[nblk]


def kernel(**inputs) -> np.ndarray:
    r = get_runner()
    in_maps = prepare_inputs(nblk=BC // NB, **inputs)
    dev = r.prepare(in_maps)
    outs = r.run(dev)
    res = r.results(outs)
    return np.concatenate([res[i]["out"][:, 0:10] for i in range(N_CORES)],
                          axis=0)
